# revision 43
# baseline (speedup 1.0000x reference)
"""Trainium2 Bass kernel for AliceAttention (dense transformer attention layer).

Reference computation (fp32):
    q/k/v = hidden @ W{q,k,v}.T  -> [B,S,NH,HD], RoPE(q,k),
    scores = q k^T / sqrt(HD) + mask, softmax, out = attn @ v,
    y = out @ Wo.T

Sharding: tensor-parallel over the 32 heads -> 4 heads per core across 8
NeuronCores. Each core computes q/k/v for its heads (columns of the
projections), full attention for its 8 (batch, head) pairs, and a partial
o_proj ( y_c = ao_c @ Wo[:, cols_c].T ); the 8 fp32 partials are summed on
the host.

Device layout choices:
  * All big matmuls run in bf16 (PE streams 1 column/cycle; fp32 is 4x
    slower). PSUM accumulation is fp32.
  * q,k are produced directly in transposed layout qT/kT = [d, t] by using
    W.T slices as the stationary operand. RoPE's rotate_half becomes a
    [128,128] +/-1 permutation matmul (P @ qT) plus elementwise combines.
  * Scores are computed transposed, scores_T = [t_k, t_q] , so that
    (a) attn @ v needs no transposes: outT[d, t_q] = v[t_k, d].T @ exp_T,
    (b) softmax denominators are a ones-column matmul over the partition
        axis, accumulated in PSUM alongside the AV matmul.
    Normalisation is deferred to after AV: outT *= (1/sums) broadcast
    across partitions via a K=1 ones matmul (float32r, exact-ish).
  * Causal masking: strictly-masked [t_k, t_q] tiles are skipped entirely;
    diagonal tiles add one of 4 precomputed [128,512] mask patterns. A
    general (non-causal) additive mask falls back to streaming mask tiles
    for every block; an all-zero mask skips masking but computes all
    blocks.
"""

import sys

import numpy as np
import ml_dtypes
from contextlib import ExitStack

import orjson

import concourse.bass as bass
import concourse.mybir as mybir
import concourse.tile as tile
import concourse.bass2jax as bass2jax
from concourse.bass_utils import run_bass_kernel_spmd

# ─────────────────────────────────────────────────────────────────────────
# This container's walrus rejects instructions carrying more semaphore
# waits than their ISA struct can hold (e.g. the Tile tail-drain with 5).
# Split excess waits into preceding wait-only EventSemaphore instructions
# (2 waits each) on the same engine — semantically identical.
# ─────────────────────────────────────────────────────────────────────────
_WAIT_CAP = {"EventSemaphore": 2}
_DEFAULT_WAIT_CAP = 1


def _legalize_bir_waits(bir_bytes: bytes) -> bytes:
    d = orjson.loads(bir_bytes)
    changed = False
    for fn in d.get("functions", []):
        for blk in fn.get("blocks", []):
            insts = blk.get("instructions")
            if not insts:
                continue
            out = []
            for inst in insts:
                si = inst.get("sync_info")
                waits = (si or {}).get("on_wait") or []
                cap = _WAIT_CAP.get(inst.get("opcode"), _DEFAULT_WAIT_CAP)
                if len(waits) > cap:
                    excess, keep = waits[:-cap], waits[-cap:]
                    for i in range(0, len(excess), 2):
                        out.append(
                            {
                                "debug": inst.get("debug"),
                                "engine": inst["engine"],
                                "ins": [],
                                "outs": [],
                                "name": f"{inst['name']}_xw{i}",
                                "opcode": "EventSemaphore",
                                "sync_info": {
                                    "on_update": [],
                                    "on_wait": excess[i : i + 2],
                                },
                            }
                        )
                    si["on_wait"] = keep
                    changed = True
                out.append(inst)
            blk["instructions"] = out
    return orjson.dumps(d) if changed else bir_bytes


if not getattr(bass2jax, "_wait_legalize_patched", False):
    _orig_compile_bir_kernel = bass2jax.compile_bir_kernel

    def _patched_compile_bir_kernel(ant_bir_str, compile_dir_path, **kw):
        return _orig_compile_bir_kernel(
            _legalize_bir_waits(ant_bir_str), compile_dir_path, **kw
        )

    bass2jax.compile_bir_kernel = _patched_compile_bir_kernel
    bass2jax._wait_legalize_patched = True

# ─────────────────────────────────────────────────────────────────────────
# Problem constants (hardcoded per contract)
# ─────────────────────────────────────────────────────────────────────────
B, S, H, NH, HD = 2, 2048, 4096, 32, 128
THETA = 10000.0
NCORES = 8
HPC = NH // NCORES          # heads per core = 4
OC = HPC * HD               # output cols per core = 512
T = B * S                   # 4096 tokens
KT = H // 128               # 32 contraction tiles for projections
TB = 512                    # t-block width in phase A
NTB = T // TB               # 8 t-blocks
NQ = S // 512               # 4 query blocks per pair
NK = S // 128               # 16 key tiles per pair
SCALE = 1.0 / float(np.sqrt(HD))

F32 = mybir.dt.float32
F32R = mybir.dt.float32r
BF16 = mybir.dt.bfloat16
BF = ml_dtypes.bfloat16
EXPF = mybir.ActivationFunctionType.Exp
LOGF = mybir.ActivationFunctionType.Ln


def _build(mode: str) -> bass.Bass:
    """mode: 'causal' (skip masked tiles, 4 diag patterns),
    'zeros' (no mask, all tiles), 'general' (stream fp32 mask tiles)."""
    nc = bass.Bass()

    xt = nc.declare_dram_parameter("xt", [H, T], BF16, isOutput=False)
    wq = nc.declare_dram_parameter("wq", [H, OC], BF16, isOutput=False)
    wk = nc.declare_dram_parameter("wk", [H, OC], BF16, isOutput=False)
    wv = nc.declare_dram_parameter("wv", [H, OC], BF16, isOutput=False)
    wo = nc.declare_dram_parameter("wo", [OC, H], BF16, isOutput=False)
    cost = nc.declare_dram_parameter("cost", [HD, T], BF16, isOutput=False)
    # sint carries rotate_half's sign: rows 0..63 hold -sin (host-prepared)
    sint = nc.declare_dram_parameter("sint", [HD, T], BF16, isOutput=False)
    ones_bf = nc.declare_dram_parameter("ones_bf", [128, 128], BF16, isOutput=False)
    if mode == "causal":
        mdiag = nc.declare_dram_parameter("mdiag", [4 * 128, 512], BF16, isOutput=False)  # 0/1 binary
    elif mode == "general":
        maskt = nc.declare_dram_parameter("maskt", [S, S], BF16, isOutput=False)  # exp(scale*mask)
    y = nc.declare_dram_parameter("y", [T, H], BF16, isOutput=True)

    # DRAM scratch (per core): roped qT/kT [OC, T] (f32r) and v [T, OC] (bf16)
    qts = nc.dram_tensor("qts", [OC, T], BF16)
    kts = nc.dram_tensor("kts", [OC, T], BF16)
    vs = nc.dram_tensor("vs", [T, OC], BF16)

    with tile.TileContext(nc) as tc, ExitStack() as octx:
        # ── pools that live for the whole kernel ──
        const_pool = octx.enter_context(tc.tile_pool(name="const", bufs=1))

        ones_sb = const_pool.tile([128, 128], BF16)
        nc.sync.dma_start(out=ones_sb[:], in_=ones_bf[:])
        if mode == "causal":
            # tile declared here; its DMA is emitted after tb0's weight/x
            # chunks so it doesn't delay the startup-critical loads
            md_sb = const_pool.tile([128, 4 * 512], BF16)
        # 4 PSUM banks reserved for phase B's sc/av pools for the WHOLE
        # kernel: phase A never touches them, so phase B's first score/av
        # matmuls carry no write-after-read hazard against phase A's
        # trailing evacuations
        rsv_pool = octx.enter_context(
            tc.tile_pool(name="psRsv", bufs=2, space="PSUM")
        )
        # first-attention-round (b=0, j=0) tiles, loaded mid-phase-A into a
        # reserved pool so round 0 starts with zero DMA wait
        warm_pool = octx.enter_context(tc.tile_pool(name="warmc", bufs=1))
        warm = [
            (
                warm_pool.tile([HD, 512], BF16, tag=f"wq{hl}", name=f"wmq{hl}"),
                warm_pool.tile([HD, 512], BF16, tag=f"wk{hl}", name=f"wmk{hl}"),
                warm_pool.tile([128, 4 * HD], BF16, tag=f"wv{hl}", name=f"wmv{hl}"),
            )
            for hl in range(HPC)
        ]

        # ═════════ Phase A: QKV projections + RoPE, spill to DRAM ═════════
        with ExitStack() as actx:
            x_pool = actx.enter_context(tc.tile_pool(name="xblk", bufs=2))
            cs_pool = actx.enter_context(tc.tile_pool(name="cosin", bufs=2))
            ev_pool = actx.enter_context(tc.tile_pool(name="evac", bufs=3))
            rp_pool = actx.enter_context(tc.tile_pool(name="rope", bufs=3))
            wv_pool = actx.enter_context(tc.tile_pool(name="wv", bufs=1))
            ps_pool = actx.enter_context(
                tc.tile_pool(name="psA", bufs=4, space="PSUM")
            )
            w_pool = actx.enter_context(tc.tile_pool(name="wqk", bufs=1))

            # weights resident: [128, k*OC + o] layouts; wq first so the
            # first accumulation can start as early as possible
            wq_sb = w_pool.tile([128, KT * OC], BF16, tag="wq")
            wk_sb = w_pool.tile([128, KT * OC], BF16, tag="wk")
            wv_sb = wv_pool.tile([128, KT * OC], BF16, tag="wv")

            for tb in range(NTB):
                tsl = slice(tb * TB, (tb + 1) * TB)
                x_sb = x_pool.tile([128, KT * TB], BF16, tag="x")
                if tb == 0:
                    # k-interleaved chunked loads, arrival order matching the
                    # accumulation chains' consumption order: the first q
                    # matmul can start after ~256KB instead of ~8.4MB, and
                    # the k-chains find wk already resident
                    for k in range(KT):
                        ksl = slice(k * 128, (k + 1) * 128)
                        nc.sync.dma_start(
                            out=wq_sb[:, k * OC : (k + 1) * OC], in_=wq[ksl, :]
                        )
                        nc.sync.dma_start(
                            out=x_sb[:, k * TB : (k + 1) * TB], in_=xt[ksl, tsl]
                        )
                        nc.sync.dma_start(
                            out=wk_sb[:, k * OC : (k + 1) * OC], in_=wk[ksl, :]
                        )
                    for k in range(KT):
                        nc.sync.dma_start(
                            out=wv_sb[:, k * OC : (k + 1) * OC],
                            in_=wv[k * 128 : (k + 1) * 128, :],
                        )
                    if mode == "causal":
                        nc.sync.dma_start(
                            out=md_sb[:].rearrange("p (r c) -> p r c", r=4),
                            in_=mdiag.rearrange("(r p) c -> p r c", p=128),
                        )
                else:
                    nc.sync.dma_start(
                        out=x_sb[:].rearrange("p (k t) -> p k t", k=KT),
                        in_=xt[:, tsl].rearrange("(k p) t -> p k t", p=128),
                    )
                cos_sb = cs_pool.tile([HD, TB], BF16, tag="cos")
                sin_sb = cs_pool.tile([HD, TB], BF16, tag="sin")
                nc.sync.dma_start(out=cos_sb[:], in_=cost[:, tsl])
                nc.sync.dma_start(out=sin_sb[:], in_=sint[:, tsl])

                # q and k: accumulate all 8 o-tiles first (dense PE), then
                # rot-matmuls read long-finished DVE copies - no PE bubbles
                raws = []
                for which, w_sb, spill in (("q", wq_sb, qts), ("k", wk_sb, kts)):
                    if tb == 0:
                        # k-major across the 4 chains: consumption (4 matmuls
                        # per 256KB chunk) is slower than DMA arrival, so the
                        # PE streams densely from the first chunk and the HAM
                        # clock stays at 2.4GHz through the startup
                        pss = [
                            ps_pool.tile(
                                [128, TB], F32, tag="proj", name=f"ps{which}{ot}"
                            )
                            for ot in range(HPC)
                        ]
                        for k in range(KT):
                            for ot in range(HPC):
                                nc.tensor.matmul(
                                    pss[ot][:],
                                    w_sb[:, k * OC + ot * 128 : k * OC + (ot + 1) * 128],
                                    x_sb[:, k * TB : (k + 1) * TB],
                                    start=(k == 0),
                                    stop=(k == KT - 1),
                                )
                        for ot in range(HPC):
                            raw_sb = ev_pool.tile(
                                [128, TB], BF16, tag="rawqk", name=f"raw{which}{ot}"
                            )
                            nc.scalar.copy(raw_sb[:], pss[ot][:])
                            raws.append((raw_sb, spill, ot))
                        continue
                    for ot in range(HPC):
                        ps = ps_pool.tile([128, TB], F32, tag="proj")
                        for k in range(KT):
                            nc.tensor.matmul(
                                ps[:],
                                w_sb[:, k * OC + ot * 128 : k * OC + (ot + 1) * 128],
                                x_sb[:, k * TB : (k + 1) * TB],
                                start=(k == 0),
                                stop=(k == KT - 1),
                            )
                        raw_sb = ev_pool.tile(
                            [128, TB], BF16, tag="rawqk", name=f"raw{which}{ot}"
                        )
                        # evacuate on ScalarE (idle in phase A): keeps the DVE
                        # queue short so phase A ends DVE-clean and phase B's
                        # first exps aren't blocked by space-reuse WARs
                        nc.scalar.copy(raw_sb[:], ps[:])
                        raws.append((raw_sb, spill, ot))
                for raw_sb, spill, ot in raws:
                    # rotate_half as an SBUF->SBUF partition-swap DMA (the
                    # sign lives in the sint table) - no PE matmul needed
                    xs_sb = ev_pool.tile([128, TB], BF16, tag="xs")
                    nc.sync.dma_start(out=xs_sb[0:64, :], in_=raw_sb[64:128, :])
                    nc.sync.dma_start(out=xs_sb[64:128, :], in_=raw_sb[0:64, :])
                    t1 = rp_pool.tile([128, TB], F32, tag="t1")
                    nc.vector.tensor_mul(t1[:], raw_sb[:], cos_sb[:])
                    t2 = rp_pool.tile([128, TB], F32, tag="t2")
                    nc.vector.tensor_mul(t2[:], xs_sb[:], sin_sb[:])
                    roped = ev_pool.tile([128, TB], BF16, tag="roped")
                    nc.vector.tensor_add(roped[:], t1[:], t2[:])
                    nc.sync.dma_start(
                        out=spill[ot * 128 : (ot + 1) * 128, tsl], in_=roped[:]
                    )

                # v: out tiles [t 128, o 512] (natural layout), spill
                def emit_v(tb, x_sb):
                    nmt = TB // 128
                    if tb == 0:
                        # k-major for the same arrival-pacing reason as above
                        pss = [
                            ps_pool.tile([128, OC], F32, tag="proj", name=f"psv{mt}")
                            for mt in range(nmt)
                        ]
                        for k in range(KT):
                            for mt in range(nmt):
                                nc.tensor.matmul(
                                    pss[mt][:],
                                    x_sb[:, k * TB + mt * 128 : k * TB + (mt + 1) * 128],
                                    wv_sb[:, k * OC : (k + 1) * OC],
                                    start=(k == 0),
                                    stop=(k == KT - 1),
                                )
                        for mt in range(nmt):
                            v_sb = ev_pool.tile([128, OC], BF16, tag="vout", name="vsb")
                            nc.scalar.copy(v_sb[:], pss[mt][:])
                            nc.sync.dma_start(
                                out=vs[tb * TB + mt * 128 : tb * TB + (mt + 1) * 128, :],
                                in_=v_sb[:],
                            )
                        return
                    for mt in range(nmt):
                        ps = ps_pool.tile([128, OC], F32, tag="proj", name="psv")
                        for k in range(KT):
                            nc.tensor.matmul(
                                ps[:],
                                x_sb[:, k * TB + mt * 128 : k * TB + (mt + 1) * 128],
                                wv_sb[:, k * OC : (k + 1) * OC],
                                start=(k == 0),
                                stop=(k == KT - 1),
                            )
                        v_sb = ev_pool.tile([128, OC], BF16, tag="vout", name="vsb")
                        nc.scalar.copy(v_sb[:], ps[:])
                        nc.sync.dma_start(
                            out=vs[tb * TB + mt * 128 : tb * TB + (mt + 1) * 128, :],
                            in_=v_sb[:],
                        )

                emit_v(tb, x_sb)

                if tb == 3:
                    # batch 0's round-0 attention tiles: their spill sources
                    # (tb0-3) are complete, load them now into the reserved
                    # warm pool so phase B round 0 has zero DMA wait
                    for hl in range(HPC):
                        osl = slice(hl * 128, (hl + 1) * 128)
                        wq_t, wk_t, wv_t = warm[hl]
                        nc.sync.dma_start(out=wk_t[:], in_=kts[osl, 0:512])
                        nc.sync.dma_start(out=wq_t[:], in_=qts[osl, 0:512])
                        nc.sync.dma_start(
                            out=wv_t[:].rearrange("p (k d) -> p k d", k=4),
                            in_=vs[0:512, osl].rearrange("(k p) d -> p k d", p=128),
                        )

        # ── residents for phases B+C (allocated after phase A frees SBUF) ──
        ao_pool = octx.enter_context(tc.tile_pool(name="ao", bufs=1))
        wo_pool = octx.enter_context(tc.tile_pool(name="wo", bufs=1))
        # attention output, transposed: one [128, T] tile per local head
        aoT = [
            ao_pool.tile([HD, T], BF16, tag=f"aoT{hl}", name=f"aoT{hl}")
            for hl in range(HPC)
        ]
        # Wo.T resident: [128, hl*H + hout]; loaded chunk-wise AFTER the
        # first attention chunks (see below) so it doesn't delay them
        wo_sb = wo_pool.tile([128, HPC * H], BF16)

        # ═════════ Phases B+C: attention + o_proj, interleaved ═════════
        # b=0 attention runs j-major across the 4 pairs (wide dep window for
        # ACT/DVE). b=1 attention is interleaved with o_proj tiles of b=0 so
        # PE stays dense while ACT works; o_proj of b=1 closes the kernel.
        with ExitStack() as bctx:
            exp_pool = bctx.enter_context(tc.tile_pool(name="exp", bufs=6))
            nrm_pool = bctx.enter_context(tc.tile_pool(name="nrm", bufs=3))
            yo_pool = bctx.enter_context(tc.tile_pool(name="yout", bufs=3))
            if mode == "general":
                mt_pool = bctx.enter_context(tc.tile_pool(name="mtile", bufs=4))
            qk_pool = bctx.enter_context(tc.tile_pool(name="qkv_pair", bufs=2))
            sc_pool = av_pool = rsv_pool  # sc/av live in the reserved banks
            sm_pool = bctx.enter_context(
                tc.tile_pool(name="psSum", bufs=1, space="PSUM")
            )
            yp_pool = bctx.enter_context(
                tc.tile_pool(name="psY", bufs=3, space="PSUM")
            )

            def alloc_pair(b, hl):
                v_sb = qk_pool.tile(
                    [128, NK * HD], BF16, tag=f"vh{hl}", name=f"v{b}{hl}", bufs=1
                )
                qT_sb = qk_pool.tile([HD, S], BF16, tag=f"qTh{hl}", name=f"qT{b}{hl}")
                kT_sb = qk_pool.tile([HD, S], BF16, tag=f"kTh{hl}", name=f"kT{b}{hl}")
                return qT_sb, kT_sb, v_sb

            def load_pair_chunk(b, hl, jc, pair):
                # one 512-token chunk of kT/qT/v for pair (b, hl): attention
                # round j only needs chunks 0..j, so emitting chunk-wise lets
                # round 0 start ~1.5MB after phase A instead of ~10MB after
                qT_sb, kT_sb, v_sb = pair
                osl = slice(hl * 128, (hl + 1) * 128)
                lsl = slice(jc * 512, (jc + 1) * 512)
                hsl = slice(b * S + jc * 512, b * S + (jc + 1) * 512)
                nc.sync.dma_start(out=kT_sb[:, lsl], in_=kts[osl, hsl])
                nc.sync.dma_start(out=qT_sb[:, lsl], in_=qts[osl, hsl])
                nc.sync.dma_start(
                    out=v_sb[:, 4 * jc * HD : (4 * jc + 4) * HD]
                    .rearrange("p (k d) -> p k d", k=4),
                    in_=vs[hsl, osl].rearrange("(k p) d -> p k d", p=128),
                )

            def emit_attn_j(b, hl, j, pair):
                qT_sb, kT_sb, v_sb = pair
                if mode == "causal":
                    kept = list(range(min(NK, 4 * j + 4)))
                else:
                    kept = list(range(NK))
                qsl = slice(j * 512, (j + 1) * 512)
                av_ps = av_pool.tile([128, 512], F32, tag="av")
                sm_ps = sm_pool.tile([128, 512], F32, tag="sm")
                use_warm = b == 0 and j == 0
                wq_t, wk_t, wv_t = warm[hl]

                def emit_sc(ki):
                    sc_ps = sc_pool.tile([128, 512], F32, tag="sc")
                    if use_warm and ki < 4:
                        k_src = wk_t[:, ki * 128 : (ki + 1) * 128]
                        q_src = wq_t[:]
                    else:
                        k_src = kT_sb[:, ki * 128 : (ki + 1) * 128]
                        q_src = qT_sb[:, qsl]
                    nc.tensor.matmul(
                        sc_ps[:], k_src, q_src, start=True, stop=True
                    )
                    exp_sb = exp_pool.tile([128, 512], BF16, tag="exp")
                    nc.scalar.activation(exp_sb[:], sc_ps[:], EXPF, scale=SCALE)
                    # multiplicative mask after exp: exp(s+m) = exp(s)*exp(m);
                    # for causal, exp(m) is exactly 0/1
                    if mode == "causal" and ki >= 4 * j:
                        r = ki - 4 * j
                        nc.vector.tensor_mul(
                            exp_sb[:], exp_sb[:], md_sb[:, r * 512 : (r + 1) * 512]
                        )
                    elif mode == "general":
                        m_sb = mt_pool.tile([128, 512], BF16, tag="mt")
                        nc.sync.dma_start(
                            out=m_sb[:], in_=maskt[ki * 128 : (ki + 1) * 128, qsl]
                        )
                        nc.vector.tensor_mul(exp_sb[:], exp_sb[:], m_sb[:])
                    return exp_sb

                def emit_av(i, ki, exp_sb):
                    if use_warm and ki < 4:
                        v_src = wv_t[:, ki * HD : (ki + 1) * HD]
                    else:
                        v_src = v_sb[:, ki * HD : (ki + 1) * HD]
                    nc.tensor.matmul(
                        av_ps[:],
                        v_src,
                        exp_sb[:],
                        start=(i == 0),
                        stop=(i == len(kept) - 1),
                    )

                # software-pipeline: sc (and its exp) run one step ahead of
                # the consuming av matmuls so ACT's exp latency is hidden.
                # Softmax denominators: DVE folds ALL exp tiles down to one
                # via a streaming binary tree (bf16 2x mode), so PE runs a
                # single ones-matmul per round instead of one per k-tile.
                pend = []
                stack = []  # (level, tile) partial fold sums

                def push_fold(t):
                    lv = 0
                    while stack and stack[-1][0] == lv:
                        _, o = stack.pop()
                        f_sb = exp_pool.tile([128, 512], BF16, tag="fold")
                        nc.vector.tensor_add(f_sb[:], o[:], t[:])
                        t = f_sb
                        lv += 1
                    stack.append((lv, t))

                for i, ki in enumerate(kept):
                    e = emit_sc(ki)
                    push_fold(e)
                    pend.append((i, ki, e))
                    if len(pend) > 1:
                        emit_av(*pend.pop(0))
                for p in pend:
                    emit_av(*p)
                while len(stack) > 1:
                    (_, a), (_, bt) = stack.pop(), stack.pop()
                    f_sb = exp_pool.tile([128, 512], BF16, tag="fold")
                    nc.vector.tensor_add(f_sb[:], a[:], bt[:])
                    stack.append((99, f_sb))
                nc.tensor.matmul(
                    sm_ps[:], ones_sb[:], stack[0][1][:], start=True, stop=True
                )
                # 1/sums on ScalarE as exp(-ln s): Ln+Exp share one ACT table
                # set, so no table reloads; keeps the 3.4us iterative divide
                # off the Vector engine (and off o_proj's critical path)
                ln_sb = nrm_pool.tile([128, 512], F32, tag="lnrm")
                nc.scalar.activation(ln_sb[:], sm_ps[:], LOGF)
                rc_sb = nrm_pool.tile([128, 512], F32, tag="rc")
                nc.scalar.activation(rc_sb[:], ln_sb[:], EXPF, scale=-1.0)
                nc.vector.tensor_mul(
                    aoT[hl][:, b * S + j * 512 : b * S + (j + 1) * 512],
                    av_ps[:],
                    rc_sb[:],
                )

            def emit_oproj_tile(b, mt, n, pool_tag=None, evac=0):
                msl = slice(b * S + mt * 128, b * S + (mt + 1) * 128)
                pool, tag = pool_tag or (yp_pool, "y")
                ps = pool.tile([128, 512], F32, tag=tag)
                for hl in range(HPC):
                    nc.tensor.matmul(
                        ps[:],
                        aoT[hl][:, msl],
                        wo_sb[:, hl * H + n * 512 : hl * H + (n + 1) * 512],
                        start=(hl == 0),
                        stop=(hl == HPC - 1),
                    )
                y_sb = yo_pool.tile([128, 512], BF16, tag="ysb")
                if evac:
                    nc.scalar.copy(y_sb[:], ps[:])
                else:
                    nc.vector.tensor_copy(y_sb[:], ps[:])
                nc.sync.dma_start(
                    out=y[msl, n * 512 : (n + 1) * 512], in_=y_sb[:]
                )

            # Per batch: after the j-th attention round (all 4 pairs), the
            # o_proj tiles for t_q in that round are ready - emit them
            # immediately so PE stays dense while ACT runs the next round's
            # exps. o_proj of round j is interleaved into round j+1.
            for b in range(B):
                pairs = [alloc_pair(b, hl) for hl in range(HPC)]
                for jc in range(NQ):
                    for hl in range(HPC):
                        load_pair_chunk(b, hl, jc, pairs[hl])
                    if b == 0 and jc == 0:
                        # wo is first needed by o_proj (after round 1 starts):
                        # load it behind the first attention chunks
                        nc.sync.dma_start(
                            out=wo_sb[:].rearrange("p (hl n) -> p hl n", hl=HPC),
                            in_=wo.rearrange("(hl p) n -> p hl n", p=128),
                        )
                ready = []
                for j in range(NQ):
                    for hl in range(HPC):
                        # fillers BEFORE the pair: their DVE evacuations queue
                        # ahead of the round's fold/mask work, so the next
                        # fillers' PSUM banks recycle without stalling PE
                        for _ in range(2 if j > 0 else 0):
                            if ready:
                                emit_oproj_tile(b, *ready.pop(0))
                        emit_attn_j(b, hl, j, pairs[hl])
                    ready.extend(
                        (mt, n)
                        for mt in range(4 * j, 4 * j + 4)
                        for n in range(H // 512)
                    )
                # trailing o_proj burst: attention's PSUM pools are idle, so
                # rotate across yp/av/sm banks (6-deep) and alternate the
                # evacuation between Vector and Scalar so neither engine's
                # queue gates the PE
                burst_pools = [(yp_pool, "y"), (av_pool, "av"), (sm_pool, "sm")]
                for t, mt_n in enumerate(ready):
                    emit_oproj_tile(b, *mt_n, pool_tag=burst_pools[t % 3])

    return nc


_CACHE: dict = {}


def _get_nc(mode: str) -> bass.Bass:
    if mode not in _CACHE:
        _CACHE[mode] = _build(mode)
    return _CACHE[mode]


def _rope_tables():
    inv_freq = 1.0 / (THETA ** (np.arange(0, HD, 2, dtype=np.float32) / HD))
    t = np.arange(S, dtype=np.float32)
    freqs = np.einsum("i,j->ij", t, inv_freq)
    emb = np.concatenate((freqs, freqs), axis=-1)  # [S, HD]
    return np.cos(emb), np.sin(emb)


def kernel(hidden_states, attention_mask, Wq, Wk, Wv, Wo):
    hs = np.asarray(hidden_states, dtype=np.float32)
    mask = np.asarray(attention_mask, dtype=np.float32)[0, 0]
    Wq = np.asarray(Wq, dtype=np.float32)
    Wk = np.asarray(Wk, dtype=np.float32)
    Wv = np.asarray(Wv, dtype=np.float32)
    Wo = np.asarray(Wo, dtype=np.float32)

    # ── mask analysis ──
    causal = np.triu(np.full((S, S), -1e9, dtype=np.float32), k=1)
    if np.array_equal(mask, causal):
        mode = "causal"
    elif not mask.any():
        mode = "zeros"
    else:
        mode = "general"

    # ── host-side prep ──
    xt = np.ascontiguousarray(hs.reshape(T, H).T).astype(BF)  # [H, T]
    cos, sin = _rope_tables()  # [S, HD] fp32
    cost = np.ascontiguousarray(np.tile(cos.T, (1, B))).astype(BF)  # [HD, T]
    # rotate_half on device is a pure partition swap; the sign of the first
    # half lives here: roped = raw*cos + swap(raw)*sinmod
    sinT = np.tile(sin.T, (1, B))
    sinmod = np.concatenate([-sinT[: HD // 2], sinT[HD // 2 :]], axis=0)
    sint = np.ascontiguousarray(sinmod).astype(BF)
    ones_bf = np.ones((128, 128), dtype=BF)

    common = {
        "cost": cost,
        "sint": sint,
        "ones_bf": ones_bf,
    }
    if mode == "causal":
        # 4 diagonal tile patterns [128, 512]: pattern r masks where
        # 128*r + p > c  (pre-scaled by sqrt(HD) since exp() applies
        # scale to mask+scores together)
        p_idx = np.arange(128)[:, None]
        c_idx = np.arange(512)[None, :]
        md = np.stack(
            [
                np.where(128 * r + p_idx > c_idx, np.float32(0.0), np.float32(1.0))
                for r in range(4)
            ]
        ).astype(BF)
        common["mdiag"] = np.ascontiguousarray(md.reshape(4 * 128, 512))
    elif mode == "general":
        common["maskt"] = np.ascontiguousarray(
            np.exp(mask.T.astype(np.float64) * SCALE)
        ).astype(BF)

    in_maps = []
    for c in range(NCORES):
        osl = slice(OC * c, OC * (c + 1))
        in_maps.append(
            dict(
                common,
                xt=xt,
                wq=np.ascontiguousarray(Wq[osl, :].T).astype(BF),
                wk=np.ascontiguousarray(Wk[osl, :].T).astype(BF),
                wv=np.ascontiguousarray(Wv[osl, :].T).astype(BF),
                wo=np.ascontiguousarray(Wo[:, osl].T).astype(BF),
            )
        )

    global _last_in_maps
    _last_in_maps = in_maps
    nc = _get_nc(mode)
    res = run_bass_kernel_spmd(nc, in_maps, list(range(NCORES)))
    out = np.zeros((T, H), dtype=np.float32)
    for c in range(NCORES):
        out += res.results[c]["y"].astype(np.float32)
    return out.reshape(B, S, H)



# revision 47
# speedup vs baseline: 1.0149x; 1.0149x over previous
"""Trainium2 Bass kernel for AliceAttention (dense transformer attention layer).

Reference computation (fp32):
    q/k/v = hidden @ W{q,k,v}.T  -> [B,S,NH,HD], RoPE(q,k),
    scores = q k^T / sqrt(HD) + mask, softmax, out = attn @ v,
    y = out @ Wo.T

Sharding: tensor-parallel over the 32 heads -> 4 heads per core across 8
NeuronCores. Each core computes q/k/v for its heads (columns of the
projections), full attention for its 8 (batch, head) pairs, and a partial
o_proj ( y_c = ao_c @ Wo[:, cols_c].T ); the 8 fp32 partials are summed on
the host.

Device layout choices:
  * All big matmuls run in bf16 (PE streams 1 column/cycle; fp32 is 4x
    slower). PSUM accumulation is fp32.
  * q,k are produced directly in transposed layout qT/kT = [d, t] by using
    W.T slices as the stationary operand. RoPE's rotate_half becomes a
    [128,128] +/-1 permutation matmul (P @ qT) plus elementwise combines.
  * Scores are computed transposed, scores_T = [t_k, t_q] , so that
    (a) attn @ v needs no transposes: outT[d, t_q] = v[t_k, d].T @ exp_T,
    (b) softmax denominators are a ones-column matmul over the partition
        axis, accumulated in PSUM alongside the AV matmul.
    Normalisation is deferred to after AV: outT *= (1/sums) broadcast
    across partitions via a K=1 ones matmul (float32r, exact-ish).
  * Causal masking: strictly-masked [t_k, t_q] tiles are skipped entirely;
    diagonal tiles add one of 4 precomputed [128,512] mask patterns. A
    general (non-causal) additive mask falls back to streaming mask tiles
    for every block; an all-zero mask skips masking but computes all
    blocks.
"""

import sys

import numpy as np
import ml_dtypes
from contextlib import ExitStack

import orjson

import concourse.bass as bass
import concourse.mybir as mybir
import concourse.tile as tile
import concourse.bass2jax as bass2jax
from concourse.bass_utils import run_bass_kernel_spmd

# ─────────────────────────────────────────────────────────────────────────
# This container's walrus rejects instructions carrying more semaphore
# waits than their ISA struct can hold (e.g. the Tile tail-drain with 5).
# Split excess waits into preceding wait-only EventSemaphore instructions
# (2 waits each) on the same engine — semantically identical.
# ─────────────────────────────────────────────────────────────────────────
_WAIT_CAP = {"EventSemaphore": 2}
_DEFAULT_WAIT_CAP = 1


def _legalize_bir_waits(bir_bytes: bytes) -> bytes:
    d = orjson.loads(bir_bytes)
    changed = False
    for fn in d.get("functions", []):
        for blk in fn.get("blocks", []):
            insts = blk.get("instructions")
            if not insts:
                continue
            out = []
            for inst in insts:
                si = inst.get("sync_info")
                waits = (si or {}).get("on_wait") or []
                cap = _WAIT_CAP.get(inst.get("opcode"), _DEFAULT_WAIT_CAP)
                if len(waits) > cap:
                    excess, keep = waits[:-cap], waits[-cap:]
                    for i in range(0, len(excess), 2):
                        out.append(
                            {
                                "debug": inst.get("debug"),
                                "engine": inst["engine"],
                                "ins": [],
                                "outs": [],
                                "name": f"{inst['name']}_xw{i}",
                                "opcode": "EventSemaphore",
                                "sync_info": {
                                    "on_update": [],
                                    "on_wait": excess[i : i + 2],
                                },
                            }
                        )
                    si["on_wait"] = keep
                    changed = True
                out.append(inst)
            blk["instructions"] = out
    return orjson.dumps(d) if changed else bir_bytes


if not getattr(bass2jax, "_wait_legalize_patched", False):
    _orig_compile_bir_kernel = bass2jax.compile_bir_kernel

    def _patched_compile_bir_kernel(ant_bir_str, compile_dir_path, **kw):
        return _orig_compile_bir_kernel(
            _legalize_bir_waits(ant_bir_str), compile_dir_path, **kw
        )

    bass2jax.compile_bir_kernel = _patched_compile_bir_kernel
    bass2jax._wait_legalize_patched = True

# ─────────────────────────────────────────────────────────────────────────
# Problem constants (hardcoded per contract)
# ─────────────────────────────────────────────────────────────────────────
B, S, H, NH, HD = 2, 2048, 4096, 32, 128
THETA = 10000.0
NCORES = 8
HPC = NH // NCORES          # heads per core = 4
OC = HPC * HD               # output cols per core = 512
T = B * S                   # 4096 tokens
KT = H // 128               # 32 contraction tiles for projections
TB = 512                    # t-block width in phase A
NTB = T // TB               # 8 t-blocks
NQ = S // 512               # 4 query blocks per pair
NK = S // 128               # 16 key tiles per pair
SCALE = 1.0 / float(np.sqrt(HD))

F32 = mybir.dt.float32
F32R = mybir.dt.float32r
BF16 = mybir.dt.bfloat16
BF = ml_dtypes.bfloat16
EXPF = mybir.ActivationFunctionType.Exp
LOGF = mybir.ActivationFunctionType.Ln


def _build(mode: str) -> bass.Bass:
    """mode: 'causal' (skip masked tiles, 4 diag patterns),
    'zeros' (no mask, all tiles), 'general' (stream fp32 mask tiles)."""
    nc = bass.Bass()

    xt = nc.declare_dram_parameter("xt", [H, T], BF16, isOutput=False)
    wq = nc.declare_dram_parameter("wq", [H, OC], BF16, isOutput=False)
    wk = nc.declare_dram_parameter("wk", [H, OC], BF16, isOutput=False)
    wv = nc.declare_dram_parameter("wv", [H, OC], BF16, isOutput=False)
    wo = nc.declare_dram_parameter("wo", [OC, H], BF16, isOutput=False)
    cost = nc.declare_dram_parameter("cost", [HD, T], BF16, isOutput=False)
    # sint carries rotate_half's sign: rows 0..63 hold -sin (host-prepared)
    sint = nc.declare_dram_parameter("sint", [HD, T], BF16, isOutput=False)
    ones_bf = nc.declare_dram_parameter("ones_bf", [128, 128], BF16, isOutput=False)
    if mode == "causal":
        mdiag = nc.declare_dram_parameter("mdiag", [4 * 128, 512], BF16, isOutput=False)  # 0/1 binary
    elif mode == "general":
        maskt = nc.declare_dram_parameter("maskt", [S, S], BF16, isOutput=False)  # exp(scale*mask)
    y = nc.declare_dram_parameter("y", [T, H], BF16, isOutput=True)

    # DRAM scratch (per core): roped qT/kT [OC, T] (f32r) and v [T, OC] (bf16)
    qts = nc.dram_tensor("qts", [OC, T], BF16)
    kts = nc.dram_tensor("kts", [OC, T], BF16)
    vs = nc.dram_tensor("vs", [T, OC], BF16)

    with tile.TileContext(nc) as tc, ExitStack() as octx:
        # ── pools that live for the whole kernel ──
        const_pool = octx.enter_context(tc.tile_pool(name="const", bufs=1))

        ones_sb = const_pool.tile([128, 128], BF16)
        nc.sync.dma_start(out=ones_sb[:], in_=ones_bf[:])
        if mode == "causal":
            # tile declared here; its DMA is emitted after tb0's weight/x
            # chunks so it doesn't delay the startup-critical loads
            md_sb = const_pool.tile([128, 4 * 512], BF16)
        # 4 PSUM banks reserved for phase B's sc/av pools for the WHOLE
        # kernel: phase A never touches them, so phase B's first score/av
        # matmuls carry no write-after-read hazard against phase A's
        # trailing evacuations
        rsv_pool = octx.enter_context(
            tc.tile_pool(name="psRsv", bufs=2, space="PSUM")
        )
        # first-attention-round (b=0, j=0) tiles, loaded mid-phase-A into a
        # reserved pool so round 0 starts with zero DMA wait
        warm_pool = octx.enter_context(tc.tile_pool(name="warmc", bufs=1))
        warm = [
            (
                warm_pool.tile([HD, 512], BF16, tag=f"wq{hl}", name=f"wmq{hl}"),
                warm_pool.tile([HD, 512], BF16, tag=f"wk{hl}", name=f"wmk{hl}"),
                warm_pool.tile([128, 4 * HD], BF16, tag=f"wv{hl}", name=f"wmv{hl}"),
            )
            for hl in range(HPC)
        ]

        # ═════════ Phase A: QKV projections + RoPE, spill to DRAM ═════════
        with ExitStack() as actx:
            x_pool = actx.enter_context(tc.tile_pool(name="xblk", bufs=2))
            cs_pool = actx.enter_context(tc.tile_pool(name="cosin", bufs=2))
            ev_pool = actx.enter_context(tc.tile_pool(name="evac", bufs=3))
            rp_pool = actx.enter_context(tc.tile_pool(name="rope", bufs=3))
            wv_pool = actx.enter_context(tc.tile_pool(name="wv", bufs=1))
            ps_pool = actx.enter_context(
                tc.tile_pool(name="psA", bufs=4, space="PSUM")
            )
            w_pool = actx.enter_context(tc.tile_pool(name="wqk", bufs=1))

            # weights resident: [128, k*OC + o] layouts; wq first so the
            # first accumulation can start as early as possible
            wq_sb = w_pool.tile([128, KT * OC], BF16, tag="wq")
            wk_sb = w_pool.tile([128, KT * OC], BF16, tag="wk")
            wv_sb = wv_pool.tile([128, KT * OC], BF16, tag="wv")

            for tb in range(NTB):
                tsl = slice(tb * TB, (tb + 1) * TB)
                x_sb = x_pool.tile([128, KT * TB], BF16, tag="x")
                if tb == 0:
                    # k-interleaved chunked loads, arrival order matching the
                    # accumulation chains' consumption order: the first q
                    # matmul can start after ~256KB instead of ~8.4MB, and
                    # the k-chains find wk already resident
                    for k in range(KT):
                        ksl = slice(k * 128, (k + 1) * 128)
                        nc.sync.dma_start(
                            out=wq_sb[:, k * OC : (k + 1) * OC], in_=wq[ksl, :]
                        )
                        nc.sync.dma_start(
                            out=x_sb[:, k * TB : (k + 1) * TB], in_=xt[ksl, tsl]
                        )
                        nc.sync.dma_start(
                            out=wk_sb[:, k * OC : (k + 1) * OC], in_=wk[ksl, :]
                        )
                    nc.sync.dma_start(
                        out=wv_sb[:].rearrange("p (k o) -> p k o", k=KT),
                        in_=wv.rearrange("(k p) o -> p k o", p=128),
                    )
                    if mode == "causal":
                        nc.sync.dma_start(
                            out=md_sb[:].rearrange("p (r c) -> p r c", r=4),
                            in_=mdiag.rearrange("(r p) c -> p r c", p=128),
                        )
                else:
                    nc.sync.dma_start(
                        out=x_sb[:].rearrange("p (k t) -> p k t", k=KT),
                        in_=xt[:, tsl].rearrange("(k p) t -> p k t", p=128),
                    )
                cos_sb = cs_pool.tile([HD, TB], BF16, tag="cos")
                sin_sb = cs_pool.tile([HD, TB], BF16, tag="sin")
                nc.sync.dma_start(out=cos_sb[:], in_=cost[:, tsl])
                nc.sync.dma_start(out=sin_sb[:], in_=sint[:, tsl])

                # q and k: accumulate all 8 o-tiles first (dense PE), then
                # rot-matmuls read long-finished DVE copies - no PE bubbles
                raws = []
                for which, w_sb, spill in (("q", wq_sb, qts), ("k", wk_sb, kts)):
                    for ot in range(HPC):
                        ps = ps_pool.tile([128, TB], F32, tag="proj")
                        for k in range(KT):
                            nc.tensor.matmul(
                                ps[:],
                                w_sb[:, k * OC + ot * 128 : k * OC + (ot + 1) * 128],
                                x_sb[:, k * TB : (k + 1) * TB],
                                start=(k == 0),
                                stop=(k == KT - 1),
                            )
                        raw_sb = ev_pool.tile(
                            [128, TB], BF16, tag="rawqk", name=f"raw{which}{ot}"
                        )
                        # evacuate on ScalarE (idle in phase A): keeps the DVE
                        # queue short so phase A ends DVE-clean and phase B's
                        # first exps aren't blocked by space-reuse WARs
                        nc.scalar.copy(raw_sb[:], ps[:])
                        raws.append((raw_sb, spill, ot))
                for raw_sb, spill, ot in raws:
                    # rotate_half as an SBUF->SBUF partition-swap DMA (the
                    # sign lives in the sint table) - no PE matmul needed
                    xs_sb = ev_pool.tile([128, TB], BF16, tag="xs")
                    nc.sync.dma_start(out=xs_sb[0:64, :], in_=raw_sb[64:128, :])
                    nc.sync.dma_start(out=xs_sb[64:128, :], in_=raw_sb[0:64, :])
                    t1 = rp_pool.tile([128, TB], F32, tag="t1")
                    nc.vector.tensor_mul(t1[:], raw_sb[:], cos_sb[:])
                    t2 = rp_pool.tile([128, TB], F32, tag="t2")
                    nc.vector.tensor_mul(t2[:], xs_sb[:], sin_sb[:])
                    roped = ev_pool.tile([128, TB], BF16, tag="roped")
                    nc.vector.tensor_add(roped[:], t1[:], t2[:])
                    nc.sync.dma_start(
                        out=spill[ot * 128 : (ot + 1) * 128, tsl], in_=roped[:]
                    )

                # v: out tiles [t 128, o 512] (natural layout), spill
                def emit_v(tb, x_sb):
                    nmt = TB // 128
                    for mt in range(nmt):
                        ps = ps_pool.tile([128, OC], F32, tag="proj", name="psv")
                        for k in range(KT):
                            nc.tensor.matmul(
                                ps[:],
                                x_sb[:, k * TB + mt * 128 : k * TB + (mt + 1) * 128],
                                wv_sb[:, k * OC : (k + 1) * OC],
                                start=(k == 0),
                                stop=(k == KT - 1),
                            )
                        v_sb = ev_pool.tile([128, OC], BF16, tag="vout", name="vsb")
                        nc.scalar.copy(v_sb[:], ps[:])
                        nc.sync.dma_start(
                            out=vs[tb * TB + mt * 128 : tb * TB + (mt + 1) * 128, :],
                            in_=v_sb[:],
                        )

                emit_v(tb, x_sb)

                if tb == 3:
                    # batch 0's round-0 attention tiles: their spill sources
                    # (tb0-3) are complete, load them now into the reserved
                    # warm pool so phase B round 0 has zero DMA wait
                    for hl in range(HPC):
                        osl = slice(hl * 128, (hl + 1) * 128)
                        wq_t, wk_t, wv_t = warm[hl]
                        nc.sync.dma_start(out=wk_t[:], in_=kts[osl, 0:512])
                        nc.sync.dma_start(out=wq_t[:], in_=qts[osl, 0:512])
                        nc.sync.dma_start(
                            out=wv_t[:].rearrange("p (k d) -> p k d", k=4),
                            in_=vs[0:512, osl].rearrange("(k p) d -> p k d", p=128),
                        )

        # ── residents for phases B+C (allocated after phase A frees SBUF) ──
        ao_pool = octx.enter_context(tc.tile_pool(name="ao", bufs=1))
        wo_pool = octx.enter_context(tc.tile_pool(name="wo", bufs=1))
        # attention output, transposed: one [128, T] tile per local head
        aoT = [
            ao_pool.tile([HD, T], BF16, tag=f"aoT{hl}", name=f"aoT{hl}")
            for hl in range(HPC)
        ]
        # Wo.T resident: [128, hl*H + hout]; loaded chunk-wise AFTER the
        # first attention chunks (see below) so it doesn't delay them
        wo_sb = wo_pool.tile([128, HPC * H], BF16)

        # ═════════ Phases B+C: attention + o_proj, interleaved ═════════
        # b=0 attention runs j-major across the 4 pairs (wide dep window for
        # ACT/DVE). b=1 attention is interleaved with o_proj tiles of b=0 so
        # PE stays dense while ACT works; o_proj of b=1 closes the kernel.
        with ExitStack() as bctx:
            exp_pool = bctx.enter_context(tc.tile_pool(name="exp", bufs=6))
            nrm_pool = bctx.enter_context(tc.tile_pool(name="nrm", bufs=3))
            yo_pool = bctx.enter_context(tc.tile_pool(name="yout", bufs=3))
            if mode == "general":
                mt_pool = bctx.enter_context(tc.tile_pool(name="mtile", bufs=4))
            qk_pool = bctx.enter_context(tc.tile_pool(name="qkv_pair", bufs=2))
            sc_pool = av_pool = rsv_pool  # sc/av live in the reserved banks
            sm_pool = bctx.enter_context(
                tc.tile_pool(name="psSum", bufs=1, space="PSUM")
            )
            yp_pool = bctx.enter_context(
                tc.tile_pool(name="psY", bufs=3, space="PSUM")
            )

            def alloc_pair(b, hl):
                v_sb = qk_pool.tile(
                    [128, NK * HD], BF16, tag=f"vh{hl}", name=f"v{b}{hl}", bufs=1
                )
                qT_sb = qk_pool.tile([HD, S], BF16, tag=f"qTh{hl}", name=f"qT{b}{hl}")
                kT_sb = qk_pool.tile([HD, S], BF16, tag=f"kTh{hl}", name=f"kT{b}{hl}")
                return qT_sb, kT_sb, v_sb

            def load_pair_chunk(b, hl, jc, pair):
                # one 512-token chunk of kT/qT/v for pair (b, hl): attention
                # round j only needs chunks 0..j, so emitting chunk-wise lets
                # round 0 start ~1.5MB after phase A instead of ~10MB after
                qT_sb, kT_sb, v_sb = pair
                osl = slice(hl * 128, (hl + 1) * 128)
                lsl = slice(jc * 512, (jc + 1) * 512)
                hsl = slice(b * S + jc * 512, b * S + (jc + 1) * 512)
                nc.sync.dma_start(out=kT_sb[:, lsl], in_=kts[osl, hsl])
                nc.sync.dma_start(out=qT_sb[:, lsl], in_=qts[osl, hsl])
                nc.sync.dma_start(
                    out=v_sb[:, 4 * jc * HD : (4 * jc + 4) * HD]
                    .rearrange("p (k d) -> p k d", k=4),
                    in_=vs[hsl, osl].rearrange("(k p) d -> p k d", p=128),
                )

            def emit_attn_j(b, hl, j, pair):
                qT_sb, kT_sb, v_sb = pair
                if mode == "causal":
                    kept = list(range(min(NK, 4 * j + 4)))
                else:
                    kept = list(range(NK))
                qsl = slice(j * 512, (j + 1) * 512)
                av_ps = av_pool.tile([128, 512], F32, tag="av")
                sm_ps = sm_pool.tile([128, 512], F32, tag="sm")
                use_warm = b == 0 and j == 0
                wq_t, wk_t, wv_t = warm[hl]

                def emit_sc(ki):
                    sc_ps = sc_pool.tile([128, 512], F32, tag="sc")
                    if use_warm and ki < 4:
                        k_src = wk_t[:, ki * 128 : (ki + 1) * 128]
                        q_src = wq_t[:]
                    else:
                        k_src = kT_sb[:, ki * 128 : (ki + 1) * 128]
                        q_src = qT_sb[:, qsl]
                    nc.tensor.matmul(
                        sc_ps[:], k_src, q_src, start=True, stop=True
                    )
                    exp_sb = exp_pool.tile([128, 512], BF16, tag="exp")
                    nc.scalar.activation(exp_sb[:], sc_ps[:], EXPF, scale=SCALE)
                    # multiplicative mask after exp: exp(s+m) = exp(s)*exp(m);
                    # for causal, exp(m) is exactly 0/1
                    if mode == "causal" and ki >= 4 * j:
                        r = ki - 4 * j
                        nc.vector.tensor_mul(
                            exp_sb[:], exp_sb[:], md_sb[:, r * 512 : (r + 1) * 512]
                        )
                    elif mode == "general":
                        m_sb = mt_pool.tile([128, 512], BF16, tag="mt")
                        nc.sync.dma_start(
                            out=m_sb[:], in_=maskt[ki * 128 : (ki + 1) * 128, qsl]
                        )
                        nc.vector.tensor_mul(exp_sb[:], exp_sb[:], m_sb[:])
                    return exp_sb

                def emit_av(i, ki, exp_sb):
                    if use_warm and ki < 4:
                        v_src = wv_t[:, ki * HD : (ki + 1) * HD]
                    else:
                        v_src = v_sb[:, ki * HD : (ki + 1) * HD]
                    nc.tensor.matmul(
                        av_ps[:],
                        v_src,
                        exp_sb[:],
                        start=(i == 0),
                        stop=(i == len(kept) - 1),
                    )

                # software-pipeline: sc (and its exp) run one step ahead of
                # the consuming av matmuls so ACT's exp latency is hidden.
                # Softmax denominators: DVE folds ALL exp tiles down to one
                # via a streaming binary tree (bf16 2x mode), so PE runs a
                # single ones-matmul per round instead of one per k-tile.
                pend = []
                stack = []  # (level, tile) partial fold sums

                def push_fold(t):
                    lv = 0
                    while stack and stack[-1][0] == lv:
                        _, o = stack.pop()
                        f_sb = exp_pool.tile([128, 512], BF16, tag="fold")
                        nc.vector.tensor_add(f_sb[:], o[:], t[:])
                        t = f_sb
                        lv += 1
                    stack.append((lv, t))

                for i, ki in enumerate(kept):
                    e = emit_sc(ki)
                    push_fold(e)
                    pend.append((i, ki, e))
                    if len(pend) > 1:
                        emit_av(*pend.pop(0))
                for p in pend:
                    emit_av(*p)
                while len(stack) > 1:
                    (_, a), (_, bt) = stack.pop(), stack.pop()
                    f_sb = exp_pool.tile([128, 512], BF16, tag="fold")
                    nc.vector.tensor_add(f_sb[:], a[:], bt[:])
                    stack.append((99, f_sb))
                nc.tensor.matmul(
                    sm_ps[:], ones_sb[:], stack[0][1][:], start=True, stop=True
                )
                # 1/sums on ScalarE as exp(-ln s): Ln+Exp share one ACT table
                # set, so no table reloads; keeps the 3.4us iterative divide
                # off the Vector engine (and off o_proj's critical path)
                ln_sb = nrm_pool.tile([128, 512], F32, tag="lnrm")
                nc.scalar.activation(ln_sb[:], sm_ps[:], LOGF)
                rc_sb = nrm_pool.tile([128, 512], F32, tag="rc")
                nc.scalar.activation(rc_sb[:], ln_sb[:], EXPF, scale=-1.0)
                nc.vector.tensor_mul(
                    aoT[hl][:, b * S + j * 512 : b * S + (j + 1) * 512],
                    av_ps[:],
                    rc_sb[:],
                )

            def emit_oproj_tile(b, mt, n, pool_tag=None, evac=0):
                msl = slice(b * S + mt * 128, b * S + (mt + 1) * 128)
                pool, tag = pool_tag or (yp_pool, "y")
                ps = pool.tile([128, 512], F32, tag=tag)
                for hl in range(HPC):
                    nc.tensor.matmul(
                        ps[:],
                        aoT[hl][:, msl],
                        wo_sb[:, hl * H + n * 512 : hl * H + (n + 1) * 512],
                        start=(hl == 0),
                        stop=(hl == HPC - 1),
                    )
                y_sb = yo_pool.tile([128, 512], BF16, tag="ysb")
                if evac:
                    nc.scalar.copy(y_sb[:], ps[:])
                else:
                    nc.vector.tensor_copy(y_sb[:], ps[:])
                nc.sync.dma_start(
                    out=y[msl, n * 512 : (n + 1) * 512], in_=y_sb[:]
                )

            # Per batch: after the j-th attention round (all 4 pairs), the
            # o_proj tiles for t_q in that round are ready - emit them
            # immediately so PE stays dense while ACT runs the next round's
            # exps. o_proj of round j is interleaved into round j+1.
            for b in range(B):
                pairs = [alloc_pair(b, hl) for hl in range(HPC)]
                for jc in range(NQ):
                    for hl in range(HPC):
                        load_pair_chunk(b, hl, jc, pairs[hl])
                    if b == 0 and jc == 0:
                        # wo is first needed by o_proj (after round 1 starts):
                        # load it behind the first attention chunks
                        nc.sync.dma_start(
                            out=wo_sb[:].rearrange("p (hl n) -> p hl n", hl=HPC),
                            in_=wo.rearrange("(hl p) n -> p hl n", p=128),
                        )
                ready = []
                for j in range(NQ):
                    for hl in range(HPC):
                        emit_attn_j(b, hl, j, pairs[hl])
                        for _ in range(2 if j > 0 else 0):
                            if ready:
                                emit_oproj_tile(b, *ready.pop(0))
                    ready.extend(
                        (mt, n)
                        for mt in range(4 * j, 4 * j + 4)
                        for n in range(H // 512)
                    )
                # trailing o_proj burst: attention's PSUM pools are idle, so
                # rotate across yp/av/sm banks (6-deep) and alternate the
                # evacuation between Vector and Scalar so neither engine's
                # queue gates the PE
                burst_pools = [(yp_pool, "y"), (av_pool, "av"), (sm_pool, "sm")]
                for t, mt_n in enumerate(ready):
                    emit_oproj_tile(b, *mt_n, pool_tag=burst_pools[t % 3])

    return nc


_CACHE: dict = {}


def _get_nc(mode: str) -> bass.Bass:
    if mode not in _CACHE:
        _CACHE[mode] = _build(mode)
    return _CACHE[mode]


def _rope_tables():
    inv_freq = 1.0 / (THETA ** (np.arange(0, HD, 2, dtype=np.float32) / HD))
    t = np.arange(S, dtype=np.float32)
    freqs = np.einsum("i,j->ij", t, inv_freq)
    emb = np.concatenate((freqs, freqs), axis=-1)  # [S, HD]
    return np.cos(emb), np.sin(emb)


def kernel(hidden_states, attention_mask, Wq, Wk, Wv, Wo):
    hs = np.asarray(hidden_states, dtype=np.float32)
    mask = np.asarray(attention_mask, dtype=np.float32)[0, 0]
    Wq = np.asarray(Wq, dtype=np.float32)
    Wk = np.asarray(Wk, dtype=np.float32)
    Wv = np.asarray(Wv, dtype=np.float32)
    Wo = np.asarray(Wo, dtype=np.float32)

    # ── mask analysis ──
    causal = np.triu(np.full((S, S), -1e9, dtype=np.float32), k=1)
    if np.array_equal(mask, causal):
        mode = "causal"
    elif not mask.any():
        mode = "zeros"
    else:
        mode = "general"

    # ── host-side prep ──
    xt = np.ascontiguousarray(hs.reshape(T, H).T).astype(BF)  # [H, T]
    cos, sin = _rope_tables()  # [S, HD] fp32
    cost = np.ascontiguousarray(np.tile(cos.T, (1, B))).astype(BF)  # [HD, T]
    # rotate_half on device is a pure partition swap; the sign of the first
    # half lives here: roped = raw*cos + swap(raw)*sinmod
    sinT = np.tile(sin.T, (1, B))
    sinmod = np.concatenate([-sinT[: HD // 2], sinT[HD // 2 :]], axis=0)
    sint = np.ascontiguousarray(sinmod).astype(BF)
    ones_bf = np.ones((128, 128), dtype=BF)

    common = {
        "cost": cost,
        "sint": sint,
        "ones_bf": ones_bf,
    }
    if mode == "causal":
        # 4 diagonal tile patterns [128, 512]: pattern r masks where
        # 128*r + p > c  (pre-scaled by sqrt(HD) since exp() applies
        # scale to mask+scores together)
        p_idx = np.arange(128)[:, None]
        c_idx = np.arange(512)[None, :]
        md = np.stack(
            [
                np.where(128 * r + p_idx > c_idx, np.float32(0.0), np.float32(1.0))
                for r in range(4)
            ]
        ).astype(BF)
        common["mdiag"] = np.ascontiguousarray(md.reshape(4 * 128, 512))
    elif mode == "general":
        common["maskt"] = np.ascontiguousarray(
            np.exp(mask.T.astype(np.float64) * SCALE)
        ).astype(BF)

    in_maps = []
    for c in range(NCORES):
        osl = slice(OC * c, OC * (c + 1))
        in_maps.append(
            dict(
                common,
                xt=xt,
                wq=np.ascontiguousarray(Wq[osl, :].T).astype(BF),
                wk=np.ascontiguousarray(Wk[osl, :].T).astype(BF),
                wv=np.ascontiguousarray(Wv[osl, :].T).astype(BF),
                wo=np.ascontiguousarray(Wo[:, osl].T).astype(BF),
            )
        )

    global _last_in_maps
    _last_in_maps = in_maps
    nc = _get_nc(mode)
    res = run_bass_kernel_spmd(nc, in_maps, list(range(NCORES)))
    out = np.zeros((T, H), dtype=np.float32)
    for c in range(NCORES):
        out += res.results[c]["y"].astype(np.float32)
    return out.reshape(B, S, H)

